# revision 23
# baseline (speedup 1.0000x reference)
"""Trainium2 Bass kernel for nn_MultiHeadAttention_47115791237226.

Computation (per token t):
  q,k,v = x @ {Wq,Wk,Wv}.T           (three 2048x2048 linears)
  reshape to (H=16, Dh=128) heads
  A[h,g] = q_h . k_g  (over Dh), causal tril mask on (h,g), softmax over g
  out[h] = sum_g A[h,g] v_g

Sharding: data-parallel over the 16384 tokens -> 2048 tokens per core, no
collectives. Linears run on the tensor engine (fp16 inputs, fp32 PSUM
accumulation). The per-token head attention is element-wise work split
across engines: QK dot products stay on the vector engine (DVE) as fused
scalar_tensor_tensor ops (that opcode fails the Pool ISA check), while
the AV accumulation chains run on gpsimd/Pool as tensor_scalar_mul +
tensor_tensor pairs. With everything on DVE the vector engine was the
bottleneck (92% busy vs PE 74%); the split drops DVE and Pool each to
~45% so the tensor engine's GEMM becomes the critical path.

Softmax skips the max-subtraction: scores are N(0, 11.3), |s| < ~60, and
exp() in fp32 neither overflows (e^60 << 3e38) nor loses the ratio's
precision. Masked entries are preset to -3e4 -> exp underflows to 0.

DMA-wait-limit notes (each DMA instruction has ONE ISA wait slot):
 - x is loaded once into a never-reused resident tile (16 fresh-range DMAs,
   zero waits).
 - W chunks stream through a bufs=2 pool on the SP/HWDGE engine as 4
   sub-DMAs per chunk; SP carries no other DMAs, so the round-robin HWDGE
   queue assignment advances exactly 8 between a slot's old and new writer
   -> same queue -> the WAW edge is program order and only the single
   PE-consumer wait remains.
 - out stores (gpsimd/SWDGE) need waits on both DVE and Pool chain tails;
   bacc's compile() splits multi-semaphore DMA waits automatically.

kernel() accepts the FULL unsharded inputs and returns the FULL output.
"""

import os
import sys

import numpy as np

sys.path.insert(0, "/opt/trn_rl_repo")

import concourse.bass as bass  # noqa: E402
import concourse.mybir as mybir  # noqa: E402
import concourse.tile as tile  # noqa: E402

# Problem constants (hardcoded per contest rules)
DIMS = 2048
HEADS = 16
HD = DIMS // HEADS  # 128
B, L = 4, 4096
TOK = B * L  # 16384
NCORES = 8
TPC = TOK // NCORES  # 2048 tokens per core
P = 128  # SBUF partitions
DC = DIMS // P  # 16 contraction chunks
OC = 256  # output-dim chunk per PSUM tile
NOC = 3 * DIMS // OC  # chunks across concat(q,k,v)
TG = 4  # token tiles per group (W is re-streamed once per group)
WSUB = 4  # sub-DMAs per W chunk; WSUB * wp-bufs must be == 0 mod 8

F16 = mybir.dt.float16
F32 = mybir.dt.float32
ALU = mybir.AluOpType
ACTF = mybir.ActivationFunctionType


# AV heads whose accumulation chain runs on DVE (fused scalar_tensor_tensor)
# instead of Pool (tensor_scalar_mul + tensor_tensor add). Pool's 2-op pair
# costs ~196 ns vs DVE's fused 194 ns (cost model), so the split is tuned to
# equalize the two engines' total load. Env knob for HW recalibration.
AV_ON_DVE = set(
    int(x) for x in os.environ.get("K_AV_DVE", "").split(",") if x.strip()
)


# GEMM output-chunk phases within a group: the kq phase interleaves k and q
# chunks (k_j, q_j, k_j+1, q_j+1, ...) so QK score pairs (which need q_h and
# k_g, g <= h) become computable progressively as chunks land, not only at
# phase end; the v phase streams v chunks in head order, which the AV
# chains consume as they land instead of piling all attention work at
# group end.
KQ_CHUNKS = []
for _j in range(NOC // 3):
    KQ_CHUNKS += [NOC // 3 + _j, _j]
V_CHUNKS = list(range(2 * NOC // 3, NOC))


def _attention(nc, pool, accp, avp, qkv, out_dram, tile_idx, tail=False):
    """Per-token-tile head attention: qkv is [P, 3*DIMS] fp16 SBUF."""
    q = qkv[:, 0:DIMS]
    k = qkv[:, DIMS : 2 * DIMS]
    v = qkv[:, 2 * DIMS : 3 * DIMS]

    # scores s[t, h*16+g] = q_h . k_g  (only g <= h computed; masked entries
    # preset to -3e4 so exp() underflows them to exactly 0).
    # scalar_tensor_tensor with accum_out: one fused DVE op per (h,g) pair.
    # (TensorScalarPtr fails the Pool-engine ISA check, so QK stays on DVE;
    # the AV side moves to Pool instead.)
    s = pool.tile([P, HEADS * HEADS], F32, tag="s")
    trash = pool.tile([P, HD], F16, tag="trash")
    nc.vector.memset(s, -30000.0)
    for h in range(1, HEADS):
        for g in range(h + 1):
            nc.vector.scalar_tensor_tensor(
                out=trash,
                in0=q[:, h * HD : (h + 1) * HD],
                scalar=1.0,
                in1=k[:, g * HD : (g + 1) * HD],
                op0=ALU.bypass,
                op1=ALU.mult,
                accum_out=s[:, h * HEADS + g : h * HEADS + g + 1],
            )

    # softmax over g without max-subtraction: P = exp(s) / sum_g exp(s),
    # computed in place so AV needs no rescale. Scores are N(0, 11.3), so
    # |s| < ~60 and fp32 exp neither overflows nor underflows the ratio.
    # Row h=0 becomes 0*inf=NaN but is never read (head 0 attends only to
    # itself: out_0 = v_0).
    s3 = s.rearrange("p (h g) -> p h g", g=HEADS)
    nc.scalar.activation(s, s, ACTF.Exp)
    sumE = pool.tile([P, HEADS], F32, tag="sumE")
    nc.vector.tensor_reduce(sumE, s3, axis=mybir.AxisListType.X, op=ALU.add)
    recip = pool.tile([P, HEADS], F32, tag="recip")
    nc.vector.reciprocal(recip, sumE)
    nc.vector.tensor_tensor(
        out=s3,
        in0=s3,
        in1=recip[:, :, None].to_broadcast((P, HEADS, HEADS)),
        op=ALU.mult,
    )

    # acc[:, h] = sum_g P[h,g] * v_g. The g=0 term is an activation-copy
    # with per-partition scale on the scalar engine; g>=1 terms run as
    # per-head serial chains on Pool (tmp = P*v on gpsimd TensorScalar,
    # then acc += tmp on gpsimd TensorTensor) or fused on DVE per AV_ON_DVE.
    acc = accp.tile([P, DIMS], F16, tag="acc")
    nc.scalar.copy(acc[:, 0:HD], v[:, 0:HD])
    for h in range(1, HEADS):
        ah = acc[:, h * HD : (h + 1) * HD]
        nc.scalar.activation(
            ah,
            v[:, 0:HD],
            ACTF.Copy,
            scale=s[:, h * HEADS : h * HEADS + 1],
        )
        # In the last group ("tail") there is no following GEMM to overlap
        # with, so split the chains across both engines (DVE is idle there);
        # odd/even split by head roughly halves the tail.
        on_dve = (h in AV_ON_DVE) or (tail and h % 2 == 1)
        for g in range(1, h + 1):
            pcol = s[:, h * HEADS + g : h * HEADS + g + 1]
            vs = v[:, g * HD : (g + 1) * HD]
            if on_dve:
                nc.vector.scalar_tensor_tensor(
                    out=ah,
                    in0=vs,
                    scalar=pcol,
                    in1=ah,
                    op0=ALU.mult,
                    op1=ALU.add,
                )
            else:
                tmp = avp.tile([P, HD], F16, tag="avtmp")
                nc.gpsimd.tensor_scalar_mul(tmp, vs, pcol)
                nc.gpsimd.tensor_tensor(out=ah, in0=ah, in1=tmp, op=ALU.add)
    nc.gpsimd.dma_start(out_dram[tile_idx * P : (tile_idx + 1) * P, :], acc)


def _body(tc, xt, wt, out, nt, reps=1):
    nc = tc.nc
    ngroups = (nt // TG) * reps
    dsub = DC // WSUB
    with (
        tc.tile_pool(name="xp", bufs=3) as xp,
        tc.tile_pool(name="wp", bufs=2) as wp,
        tc.tile_pool(name="qkvp", bufs=2) as qkvp,
        tc.tile_pool(name="psum", bufs=8, space="PSUM") as pp,
        tc.tile_pool(name="attn", bufs=9) as attnp,
        tc.tile_pool(name="accp", bufs=6) as accp,
        tc.tile_pool(name="avp", bufs=8) as avp,
    ):
        # Phase schedule: each group normally runs (kq, v) back-to-back. The
        # LAST group is split into two half-width sub-phases
        # (kq[0:2], kq[2:4], v[0:2], v[2:4]) sharing one qkv buffer: its
        # scores complete well before PE finishes, the two AV waves stream
        # behind the remaining v GEMM, and the W chunks for the extra
        # sub-phases are simply re-streamed (+25 MB, far under DMA budget).
        # The last group's AV chains are split across DVE+Pool ("tail")
        # since no further GEMM hides them.
        HT = TG // 2
        sched = []
        for gi_r in range(ngroups - 1):
            sched += [(gi_r, "kq", 0, TG), (gi_r, "v", 0, TG)]
        g_last = ngroups - 1
        sched += [
            (g_last, "kq", 0, HT),
            (g_last, "v", 0, HT),
            (g_last, "kq", HT, TG),
            (g_last, "v", HT, TG),
        ]

        qkv_tiles = {}
        x_tiles = {}
        for gi_r, phase, t_lo, t_hi in sched:
            gi = gi_r % (nt // TG)
            if phase == "kq" and t_lo == 0:
                qkv_new = qkvp.tile([P, TG, 3 * DIMS], F16, tag="qkv")
                qkv_tiles[gi_r] = qkv_new
                xg_new = xp.tile([P, TG, DC * P], F16, tag="xg")
                for t in range(TG):
                    nc.gpsimd.dma_start(xg_new[:, t, :], xt[gi * TG + t])
                x_tiles[gi_r] = xg_new
            qkv = qkv_tiles[gi_r]
            xg = x_tiles[gi_r]
            for oc in KQ_CHUNKS if phase == "kq" else V_CHUNKS:
                w = wp.tile([P, DC, OC], F16, tag="w")
                for sub in range(WSUB):
                    nc.sync.dma_start(
                        w[:, sub * dsub : (sub + 1) * dsub, :],
                        wt[:, sub * dsub : (sub + 1) * dsub, oc * OC : (oc + 1) * OC],
                    )
                for t in range(t_lo, t_hi):
                    ps = pp.tile([P, OC], F32, tag="ps")
                    for d in range(DC):
                        nc.tensor.matmul(
                            ps,
                            lhsT=xg[:, t, d * P : (d + 1) * P],
                            rhs=w[:, d, :],
                            start=(d == 0),
                            stop=(d == DC - 1),
                        )
                    nc.scalar.copy(qkv[:, t, oc * OC : (oc + 1) * OC], ps)
            if phase == "v":
                # Only the final sub-phase's AV is split across DVE+Pool:
                # earlier tiles' AV hides under remaining GEMM on Pool alone,
                # and keeping it off DVE lets the last tiles' QK start the
                # moment their k,q chunks land.
                tail = gi_r == ngroups - 1 and t_lo > 0
                for t in range(t_lo, t_hi):
                    _attention(
                        nc, attnp, accp, avp, qkv[:, t], out, gi * TG + t, tail=tail
                    )
                if t_hi == TG:
                    del qkv_tiles[gi_r]
                    del x_tiles[gi_r]


def build(tpc=TPC, reps=1):
    import concourse.bacc as bacc

    # Bacc (not raw Bass): its compile() pass splits multi-semaphore DMA
    # waits that the one-wait-slot DMA ISA encoding cannot carry.
    nc = bacc.Bacc(
        None,
        target_bir_lowering=False,
        debug=False,
        enable_asserts=True,
        num_devices=NCORES,
    )
    nt = tpc // P
    xt = nc.dram_tensor("xt", [nt, P, DC * P], F16, kind="ExternalInput").ap()
    wt = nc.dram_tensor("wt", [P, DC, 3 * DIMS], F16, kind="ExternalInput").ap()
    out = nc.dram_tensor("out", [tpc, DIMS], F16, kind="ExternalOutput").ap()
    with tile.TileContext(nc) as tc:
        _body(tc, xt, wt, out, nt, reps=reps)
    nc.compile()
    return nc


def prep_inputs(input_seq_embs, Wq, Wk, Wv, tpc=TPC, ncores=NCORES):
    """Host-side sharding + layout."""
    x = np.asarray(input_seq_embs, dtype=np.float32).reshape(TOK, DIMS)
    wall = np.concatenate(
        [np.asarray(Wq), np.asarray(Wk), np.asarray(Wv)], axis=0
    ).astype(np.float32)  # [3*DIMS, DIMS], row o, col d
    # wt[p, d, o] = wall[o, d*P+p]
    wt = np.ascontiguousarray(
        wall.T.reshape(DC, P, 3 * DIMS).transpose(1, 0, 2)
    ).astype(np.float16)
    in_maps = []
    for c in range(ncores):
        xs = x[c * tpc : (c + 1) * tpc]
        nt = tpc // P
        # xtile[t, p, d*P+q] = xs[t*P+q, d*P+p]
        xtile = (
            xs.reshape(nt, P, DC, P).transpose(0, 3, 2, 1).astype(np.float16)
        ).reshape(nt, P, DC * P)
        in_maps.append({"xt": np.ascontiguousarray(xtile), "wt": wt})
    return in_maps


_cached = {}


def _get_nc():
    if "nc" not in _cached:
        _cached["nc"] = build()
    return _cached["nc"]


def kernel_with_results(**inputs):
    from concourse import bass_utils

    nc = _get_nc()
    in_maps = prep_inputs(**inputs)
    trace = bool(int(os.environ.get("KERNEL_TRACE", "0")))
    if trace:
        try:  # NTFF profiling hook is absent in some containers
            from antenv.axon_hooks import get_axon_ntff_profile_hook  # noqa: F401
        except ImportError:
            trace = False
    res = bass_utils.run_bass_kernel_spmd(
        nc,
        in_maps,
        core_ids=list(range(NCORES)),
        trace=trace,
        trace_cores=[0] if trace else None,
    )
    outs = [r["out"] for r in res.results]
    full = (
        np.concatenate(outs, axis=0)
        .astype(np.float32)
        .reshape(B, L, DIMS)
    )
    return full, res


def kernel(**inputs):
    return kernel_with_results(**inputs)[0]
